# revision 1
# baseline (speedup 1.0000x reference)
"""Guided filter (radius=3) on 8x TRN2 NeuronCores, batch-parallel.

Per core: one image. Box filters = banded matmuls on the PE:
  pass A1: lhsT = image block (stationary), rhs = vertical band -> (w, h') transposed, PSUM-accumulated
  pass A2: lhsT = horizontal band (stationary), rhs = A1 evac    -> (w', h') natural-per-axis
Stage-2 (boxes of a, b) repeats the pair, returning to natural layout.
Band weights are exactly 1/8 (bf16-exact); the 64/49 normalization is folded
into evac copies so box results stay exact-scale in fp32 PSUM.

I and p are resident in SBUF as bf16 (cast during SWDGE DMA, large
contiguous descriptors across all 16 SDMA engines); strips slice them via
access patterns, so no per-strip scatter DMAs. q is staged f32 and stored
via SWDGE (the two HWDGE rings only reach ~50 GB/s combined).
"""

import sys

sys.path.insert(0, "/opt/trn_rl_repo")

import numpy as np
import ml_dtypes

R = 3
H = W = 1024
P = 128
NC_N = 8
V = 122  # valid outputs per 128-wide band matmul
S = float(64.0 / 49.0)

_cache = {}


def _strips():
    # (in_lo, in_hi, out_lo, out_hi) along one axis
    out = []
    j = 0
    while j * V < W:
        o_lo, o_hi = j * V, min(W, j * V + V)
        i_lo, i_hi = max(0, o_lo - R), min(W, o_hi + R)
        out.append((i_lo, i_hi, o_lo, o_hi))
        j += 1
    return out


def _band7_np():
    b = np.zeros((128, 134), np.float32)
    for k in range(128):
        for d in range(134):
            if abs(d - 3 - k) <= R:
                b[k, d] = 0.125
    return b.astype(ml_dtypes.bfloat16)


def _bandm_np(i_lo, i_hi, o_lo, o_hi):
    K = i_hi - i_lo
    bm = np.zeros((K, 128), np.float32)
    for k in range(K):
        for m in range(o_hi - o_lo):
            if abs((i_lo + k) - (o_lo + m)) <= R:
                bm[k, m] = 0.125
    return bm.astype(ml_dtypes.bfloat16)


def _seg512(lo, hi):
    """split [lo,hi) at multiples of 512 (PSUM bank boundaries)"""
    segs = []
    while lo < hi:
        nxt = min(hi, (lo // 512 + 1) * 512)
        segs.append((lo, nxt))
        lo = nxt
    return segs


def _build():
    import concourse.bass as bass
    import concourse.bacc as bacc
    import concourse.mybir as mybir
    from concourse import tile

    bf16 = mybir.dt.bfloat16
    f32 = mybir.dt.float32
    Copy = mybir.ActivationFunctionType.Copy
    Square = mybir.ActivationFunctionType.Square
    Alu = mybir.AluOpType

    strips = _strips()
    NS = len(strips)

    nc = bacc.Bacc(None, target_bir_lowering=False)
    dI = nc.dram_tensor("I", [H, W], f32, kind="ExternalInput")
    dp = nc.dram_tensor("p", [3, H, W], f32, kind="ExternalInput")
    db7 = nc.dram_tensor("band7", [128, 134], bf16, kind="ExternalInput")
    dbm_f = nc.dram_tensor("bandm_first", [125, 128], bf16, kind="ExternalInput")
    dbm_i = nc.dram_tensor("bandm_int", [128, 128], bf16, kind="ExternalInput")
    dbm_l = nc.dram_tensor("bandm_last", [51, 128], bf16, kind="ExternalInput")
    dq = nc.dram_tensor("q", [3, H, W], f32, kind="ExternalOutput")

    with tile.TileContext(nc) as tc:
        with (
            tc.tile_pool(name="const", bufs=1) as constp,
            tc.tile_pool(name="resid", bufs=1) as residp,
            tc.tile_pool(name="pbuf", bufs=1) as pbufp,
            tc.tile_pool(name="prod", bufs=1) as prodp,
            tc.tile_pool(name="vt", bufs=4) as vtp,
            tc.tile_pool(name="uirv", bufs=1) as uirvp,
            tc.tile_pool(name="tmp16", bufs=2) as t16p,
            tc.tile_pool(name="tmp32", bufs=1) as t32p,
            tc.tile_pool(name="ab", bufs=2) as abp,
            tc.tile_pool(name="fin", bufs=2) as finp,
            tc.tile_pool(name="psA", bufs=2, space="PSUM") as psA,
            tc.tile_pool(name="psB", bufs=2, space="PSUM") as psB,
        ):
            band7 = constp.tile([128, 134], bf16, tag="band7")
            nc.sync.dma_start(band7[:], db7.ap()[:])
            bm_first = constp.tile([125, 128], bf16, tag="bmf")
            nc.sync.dma_start(bm_first[:], dbm_f.ap()[:])
            bm_int = constp.tile([128, 128], bf16, tag="bmi")
            nc.sync.dma_start(bm_int[:], dbm_i.ap()[:])
            bm_last = constp.tile([51, 128], bf16, tag="bml")
            nc.sync.dma_start(bm_last[:], dbm_l.ap()[:])

            # Resident inputs, cast f32->bf16 during SWDGE DMA. Block layout:
            # tile[p, i*1024 + w] = X[i*128 + p, w] -- per-(p,i) 4KB contiguous reads.
            # 128-partition APs only: SWDGE sprays those across all 16 SDMA
            # engines; odd partition counts land on 2 engines (~26 GB/s).
            I_bf = residp.tile([128, 8 * 1024], bf16, tag="I_bf")
            nc.gpsimd.dma_start(
                I_bf[:].rearrange("p (i w) -> p i w", w=1024),
                dI.ap().rearrange("(i p) w -> p i w", p=128),
            )
            p_bf = pbufp.tile([128, 8 * 1024], bf16, tag="p_bf")
            nc.gpsimd.dma_start(
                p_bf[:].rearrange("p (i w) -> p i w", w=1024),
                dp.ap()[0].rearrange("(i p) w -> p i w", p=128),
            )
            # Strip layout for the final combine: I_nat[j, m*1024 + w] = I[122m + j, w]
            # Overlapping 128-row windows at stride 122 keep the AP 128-partition.
            from concourse.ap import AP as _AP

            I_nat = residp.tile([128, NS * 1024], bf16, tag="I_nat")
            src = dI.ap()
            src_ov = _AP(src.tensor, 0, [[1024, 128], [V * 1024, 8], [1, 1024]])
            nc.gpsimd.dma_start(
                I_nat[:, 0 : 8 * 1024].rearrange("p (m w) -> p m w", w=1024), src_ov
            )
            nc.gpsimd.dma_start(
                I_nat[0 : H - 8 * V, 8 * 1024 : 9 * 1024], dI.ap()[8 * V : H, :]
            )

            def bandm_for(si):
                if si == 0:
                    return bm_first
                if si == NS - 1:
                    return bm_last
                return bm_int

            def a1_pass(ps, tile_, off0, stride, Mw):
                """vertical box + transpose: accumulate 8 h-blocks into ps[0:Mw, 0:1024].

                lhsT for block i = tile_[:, i*stride + off0 : i*stride + off0 + Mw].
                start=True clears has_written for the WHOLE psum bank, so only
                the first matmul touching each 512-wide bank may use it.
                """
                seen = set()
                for i in range(8):
                    lhsT = tile_[:, i * stride + off0 : i * stride + off0 + Mw]
                    base = 128 * i - 3
                    w_lo_ = max(0, 128 * i - 3)
                    w_hi_ = min(1024, 128 * i + 131)
                    for s_lo, s_hi in _seg512(w_lo_, w_hi_):
                        bank = s_lo // 512
                        nc.tensor.matmul(
                            ps[0:Mw, s_lo:s_hi],
                            lhsT,
                            band7[:, s_lo - base : s_hi - base],
                            start=bank not in seen,
                            stop=True,
                        )
                        seen.add(bank)

            def a2_pass(ps, vt_tile, si):
                """horizontal box via band-stationary matmul: ps[0:128, 0:1024]"""
                i_lo, i_hi, o_lo, o_hi = strips[si]
                K = i_hi - i_lo
                bm = bandm_for(si)
                for s_lo, s_hi in _seg512(0, 1024):
                    nc.tensor.matmul(
                        ps[:, s_lo:s_hi],
                        bm[:],
                        vt_tile[0:K, s_lo:s_hi],
                        start=True,
                        stop=True,
                    )

            I3 = I_bf[:].rearrange("p (i w) -> p i w", w=1024)

            # ---------------- phase A: per strip, boxes of I and I*I -> uI, rv
            uI_tiles = []
            rv_tiles = []

            def emit_A(si):
                i_lo, i_hi, o_lo, o_hi = strips[si]
                Mw = i_hi - i_lo
                K_out = o_hi - o_lo

                ii_s = prodp.tile([128, 8 * Mw], bf16, tag="prodA")
                nc.vector.tensor_mul(
                    ii_s[:].rearrange("p (i w) -> p i w", w=Mw),
                    I3[:, :, i_lo:i_hi],
                    I3[:, :, i_lo:i_hi],
                )

                # boxes I and II: a1 passes back-to-back (no PE FIFO stall)
                psa = psA.tile([128, 1024], f32, tag="psa")
                a1_pass(psa, I_bf, i_lo, 1024, Mw)
                psa2 = psA.tile([128, 1024], f32, tag="psa")
                a1_pass(psa2, ii_s, 0, Mw, Mw)
                vt = vtp.tile([128, 1024], bf16, tag="vt")
                nc.scalar.activation(vt[0:Mw, :], psa[0:Mw, :], Copy, bias=0.0, scale=1.0)
                vt2 = vtp.tile([128, 1024], bf16, tag="vt")
                nc.scalar.activation(vt2[0:Mw, :], psa2[0:Mw, :], Copy, bias=0.0, scale=1.0)
                psu = psB.tile([128, 1024], f32, tag="psb")
                a2_pass(psu, vt, si)
                psu2 = psB.tile([128, 1024], f32, tag="psb")
                a2_pass(psu2, vt2, si)
                uI_t = uirvp.tile([128, 1024], bf16, tag=f"uI_{si}")
                nc.scalar.activation(uI_t[0:K_out, :], psu[0:K_out, :], Copy, bias=0.0, scale=S)

                # var = S*uII - uI^2 ; rv = 1/var (approx)
                sq = t16p.tile([128, 1024], bf16, tag="q12")
                nc.vector.tensor_mul(sq[0:K_out, :], uI_t[0:K_out, :], uI_t[0:K_out, :])
                var_t = t32p.tile([128, 1024], f32, tag="var")
                nc.vector.scalar_tensor_tensor(
                    var_t[0:K_out, :], psu2[0:K_out, :], S, sq[0:K_out, :],
                    Alu.mult, Alu.subtract,
                )
                rv_t = t32p.tile([128, 1024], f32, tag="rv")
                nc.vector.reciprocal_approx_fast(rv_t[0:K_out, :], var_t[0:K_out, :])
                rv_bf = uirvp.tile([128, 1024], bf16, tag=f"rv_{si}")
                nc.scalar.activation(rv_bf[0:K_out, :], rv_t[0:K_out, :], Copy, bias=0.0, scale=1.0)
                uI_tiles.append(uI_t)
                rv_tiles.append(rv_bf)

            # ---------------- phase B + stage-2, per channel
            def b1_pass(ps, tiles, m_lo, m_hi):
                """H-box of a/b over w'-strips; out ps[0:(m_hi-m_lo), 0:1024]"""
                seen = set()
                for sj, (ji_lo, ji_hi, jo_lo, jo_hi) in enumerate(strips):
                    K = jo_hi - jo_lo
                    lhsT = tiles[sj][0:K, m_lo:m_hi]
                    base = jo_lo - 3
                    w_lo_ = max(0, jo_lo - 3)
                    w_hi_ = min(1024, jo_lo + 125)
                    for s_lo, s_hi in _seg512(w_lo_, w_hi_):
                        bank = s_lo // 512
                        nc.tensor.matmul(
                            ps[0 : m_hi - m_lo, s_lo:s_hi],
                            lhsT,
                            band7[0:K, s_lo - base : s_hi - base],
                            start=bank not in seen,
                            stop=True,
                        )
                        seen.add(bank)

            def emit_B_strip(c, si, p_tile, p3, a_tiles, b_tiles):
                i_lo, i_hi, o_lo, o_hi = strips[si]
                Mw = i_hi - i_lo
                K_out = o_hi - o_lo

                ip_s = prodp.tile([128, 8 * Mw], bf16, tag="prodB")
                nc.vector.tensor_mul(
                    ip_s[:].rearrange("p (i w) -> p i w", w=Mw),
                    I3[:, :, i_lo:i_hi],
                    p3[:, :, i_lo:i_hi],
                )

                # box p + box Ip: both a1 passes back-to-back so the PE
                # never stalls behind a bridge evac in its FIFO.
                psa3 = psA.tile([128, 1024], f32, tag="psa")
                a1_pass(psa3, p_tile, i_lo, 1024, Mw)
                psa4 = psA.tile([128, 1024], f32, tag="psa")
                a1_pass(psa4, ip_s, 0, Mw, Mw)
                vt3 = vtp.tile([128, 1024], bf16, tag="vt")
                nc.scalar.activation(vt3[0:Mw, :], psa3[0:Mw, :], Copy, bias=0.0, scale=1.0)
                vt4 = vtp.tile([128, 1024], bf16, tag="vt")
                nc.scalar.activation(vt4[0:Mw, :], psa4[0:Mw, :], Copy, bias=0.0, scale=1.0)
                psu3 = psB.tile([128, 1024], f32, tag="psb")
                a2_pass(psu3, vt3, si)
                psu4 = psB.tile([128, 1024], f32, tag="psb")
                a2_pass(psu4, vt4, si)
                up_bf = t16p.tile([128, 1024], bf16, tag="up")
                nc.scalar.activation(up_bf[0:K_out, :], psu3[0:K_out, :], Copy, bias=0.0, scale=S)

                # a = (S*uIp - uI*up) * rv ; b = up - a*uI
                # m1 straight from PSUM ((psu3*S)*uI) so vector doesn't wait
                # for the scalar up-evac on the critical path.
                m1 = t16p.tile([128, 1024], bf16, tag="mm")
                nc.vector.scalar_tensor_tensor(
                    m1[0:K_out, :], psu3[0:K_out, :], S, uI_tiles[si][0:K_out, :],
                    Alu.mult, Alu.mult,
                )
                cov = t16p.tile([128, 1024], bf16, tag="cov")
                nc.vector.scalar_tensor_tensor(
                    cov[0:K_out, :], psu4[0:K_out, :], S, m1[0:K_out, :],
                    Alu.mult, Alu.subtract,
                )
                a_t = abp.tile([128, 1024], bf16, tag=f"a_{si}")
                nc.vector.tensor_mul(a_t[0:K_out, :], cov[0:K_out, :], rv_tiles[si][0:K_out, :])
                m2 = t16p.tile([128, 1024], bf16, tag="mm")
                nc.vector.tensor_mul(m2[0:K_out, :], a_t[0:K_out, :], uI_tiles[si][0:K_out, :])
                b_t = abp.tile([128, 1024], bf16, tag=f"b_{si}")
                nc.vector.tensor_sub(b_t[0:K_out, :], up_bf[0:K_out, :], m2[0:K_out, :])
                a_tiles.append(a_t)
                b_tiles.append(b_t)

            def emit_S2_m(c, m, a_tiles, b_tiles):
                mi_lo, mi_hi, mo_lo, mo_hi = strips[m]
                Hw = mo_hi - mo_lo
                Mi = mi_hi - mi_lo
                psc_a = psA.tile([128, 1024], f32, tag="psa")
                b1_pass(psc_a, a_tiles, mi_lo, mi_hi)
                abox = vtp.tile([128, 1024], bf16, tag="vt")
                nc.scalar.activation(abox[0:Mi, :], psc_a[0:Mi, :], Copy, bias=0.0, scale=S)
                psc_b = psA.tile([128, 1024], f32, tag="psa")
                b1_pass(psc_b, b_tiles, mi_lo, mi_hi)
                bbox = vtp.tile([128, 1024], bf16, tag="vt")
                nc.vector.tensor_scalar_mul(bbox[0:Mi, :], psc_b[0:Mi, :], S)

                bm2 = bandm_for(m)
                psd_a = psB.tile([128, 1024], f32, tag="psb")
                psd_b = psB.tile([128, 1024], f32, tag="psb")
                for s_lo, s_hi in _seg512(0, 1024):
                    nc.tensor.matmul(psd_a[:, s_lo:s_hi], bm2[:], abox[0:Mi, s_lo:s_hi], start=True, stop=True)
                for s_lo, s_hi in _seg512(0, 1024):
                    nc.tensor.matmul(psd_b[:, s_lo:s_hi], bm2[:], bbox[0:Mi, s_lo:s_hi], start=True, stop=True)

                qa = t16p.tile([128, 1024], bf16, tag="qa")
                nc.scalar.activation(qa[0:Hw, :], psd_a[0:Hw, :], Copy, bias=0.0, scale=1.0)
                qb = t16p.tile([128, 1024], bf16, tag="qb")
                nc.scalar.activation(qb[0:Hw, :], psd_b[0:Hw, :], Copy, bias=0.0, scale=1.0)

                q1 = t16p.tile([128, 1024], bf16, tag="q12")
                nc.vector.tensor_mul(
                    q1[0:Hw, :], qa[0:Hw, :], I_nat[0:Hw, m * 1024 : m * 1024 + 1024]
                )
                q2 = t16p.tile([128, 1024], bf16, tag="q12")
                nc.gpsimd.tensor_add(q2[0:Hw, :], q1[0:Hw, :], qb[0:Hw, :])
                q3 = finp.tile([128, 1024], bf16, tag="tt")
                nc.gpsimd.tensor_scalar(q3[0:Hw, :], q2[0:Hw, :], 1.0, 0.0, Alu.min, Alu.max)
                nc.gpsimd.dma_start(dq.ap()[c][mo_lo:mo_hi, :], q3[0:Hw, :])

            # Interleave: B(c) strip si alongside S2(c-1) output strip m=si,
            # so two independent pipelines keep every engine fed.
            for si in range(NS):
                emit_A(si)
            p3_0 = p_bf[:].rearrange("p (i w) -> p i w", w=1024)
            ab0_a = []
            ab0_b = []
            for si in range(NS):
                emit_B_strip(0, si, p_bf, p3_0, ab0_a, ab0_b)
            ab_saved = {0: (ab0_a, ab0_b)}
            for c in (1, 2):
                p_next = pbufp.tile([128, 8 * 1024], bf16, tag="p_bf")
                nc.gpsimd.dma_start(
                    p_next[:].rearrange("p (i w) -> p i w", w=1024),
                    dp.ap()[c].rearrange("(i p) w -> p i w", p=128),
                )
                p3 = p_next[:].rearrange("p (i w) -> p i w", w=1024)
                a_tiles = []
                b_tiles = []
                for si in range(NS):
                    emit_B_strip(c, si, p_next, p3, a_tiles, b_tiles)
                    emit_S2_m(c - 1, si, *ab_saved[c - 1])
                ab_saved[c] = (a_tiles, b_tiles)
            for m in range(NS):
                emit_S2_m(2, m, *ab_saved[2])

    nc.compile()
    return nc


def kernel(I, p, radius):
    assert int(radius) == R
    I = np.ascontiguousarray(np.asarray(I, np.float32))
    p = np.ascontiguousarray(np.asarray(p, np.float32))
    B = I.shape[0]
    assert I.shape == (B, 1, H, W) and p.shape == (B, 3, H, W)

    if "nc" not in _cache:
        _cache["nc"] = _build()
    nc = _cache["nc"]

    from concourse.bass_utils import run_bass_kernel_spmd

    b7 = _band7_np()
    strips = _strips()
    bm_f = _bandm_np(*strips[0])
    bm_i = _bandm_np(*strips[1])
    bm_l = _bandm_np(*strips[-1])

    in_maps = []
    for i in range(B):
        in_maps.append(
            {
                "I": I[i, 0],
                "p": p[i],
                "band7": b7,
                "bandm_first": bm_f,
                "bandm_int": bm_i,
                "bandm_last": bm_l,
            }
        )
    res = run_bass_kernel_spmd(nc, in_maps, core_ids=list(range(B)))
    out = np.stack([res.results[i]["q"] for i in range(B)], axis=0)
    return out.astype(np.float32)



# revision 7
# speedup vs baseline: 1.0202x; 1.0202x over previous
"""Guided filter (radius=3) on 8x TRN2 NeuronCores, batch-parallel. v2.

Per core: one image. Box filters = banded matmuls on the PE (exactly one
layout-swap pass + one layout-keep pass per separable box; 28 passes/image).

v2 vs baseline:
  - Custom fused DVE ops: SCALE_SUB_SQ (var = S*uII_psum - uI^2, kills the
    separate square), CLAMP01_ADDSC (q = clip(S*(Q1*I + Q2)), kills the
    evac+add+clamp chain -- the combine reads both PSUM keeps directly).
  - Engine rebalance: ACT owns all PSUM bridge evacuations (1.11us/tile),
    Pool owns the bf16 products (I*I, I*p) + S2 B-copies + q DMA issue,
    DVE owns the PSUM-math (var/cov/recip/q-combine) + bf16 chain ops.
  - Stage-2 bridge evacs are pure copies (the 64/49 normalization is folded
    into the final fused clamp), so Pool's copy path can carry them.
  - Channel-pipelined emission: S1 of channel c+1 interleaves with S2 of
    channel c (a/b strip pools sized 15 = 9 + 6 lookahead).
"""

import sys

sys.path.insert(0, "/opt/trn_rl_repo")

import numpy as np
import ml_dtypes

R = 3
H = W = 1024
P = 128
V = 122  # valid outputs per 128-wide band matmul
S = float(64.0 / 49.0)

_cache = {}


# ---------------------------------------------------------------- custom DVE
def _register_ops():
    from concourse.dve_ops import DveOp, OPS, _SUB_OPCODE_FOR_NAME
    from concourse.dve_spec import (
        Spec, Src0, Src1, C0, Zero, One, maxx, minn, sq, lower,
    )
    from concourse.dve_uop import DveOpSpec

    def reg(name, spec):
        existing = {op.name: op for op in OPS}
        if name in existing:
            return existing[name]
        opcode = max(_SUB_OPCODE_FOR_NAME.values()) + 1
        assert opcode < 0x20, "custom-DVE opcode table full"
        shas = {}
        for ver in ("v3", "v4"):
            s = DveOpSpec(name=name, opcode=opcode, uops=lower(spec, ver=ver))
            shas[ver] = s.sha(ver)
        op = DveOp(name, spec, subdim=False, uops_sha=shas)
        OPS.append(op)
        _SUB_OPCODE_FOR_NAME[name] = opcode
        return op

    clamp = reg(
        "CLAMP01_ADDSC_ANT",
        Spec(
            body=minn(maxx((Src0 + Src1) * C0, Zero), One),
            reference=lambda in0, in1, s0, s1, imm2: np.clip(
                (in0 + in1) * s0, 0.0, 1.0
            ),
        ),
    )
    var = reg(
        "SCALE_SUB_SQ_ANT",
        Spec(
            body=Src0 * C0 - sq(Src1),
            reference=lambda in0, in1, s0, s1, imm2: in0 * s0 - in1 * in1,
        ),
    )
    return clamp, var


def _strips():
    # (in_lo, in_hi, out_lo, out_hi) along one axis
    out = []
    j = 0
    while j * V < W:
        o_lo, o_hi = j * V, min(W, j * V + V)
        i_lo, i_hi = max(0, o_lo - R), min(W, o_hi + R)
        out.append((i_lo, i_hi, o_lo, o_hi))
        j += 1
    return out


def _band7_np():
    b = np.zeros((128, 134), np.float32)
    for k in range(128):
        for d in range(134):
            if abs(d - 3 - k) <= R:
                b[k, d] = 0.125
    return b.astype(ml_dtypes.bfloat16)


def _bandm_np(i_lo, i_hi, o_lo, o_hi):
    K = i_hi - i_lo
    bm = np.zeros((K, 128), np.float32)
    for k in range(K):
        for m in range(o_hi - o_lo):
            if abs((i_lo + k) - (o_lo + m)) <= R:
                bm[k, m] = 0.125
    return bm.astype(ml_dtypes.bfloat16)


def _seg512(lo, hi):
    """split [lo,hi) at multiples of 512 (PSUM bank boundaries)"""
    segs = []
    while lo < hi:
        nxt = min(hi, (lo // 512 + 1) * 512)
        segs.append((lo, nxt))
        lo = nxt
    return segs


def _build():
    import concourse.bass as bass
    import concourse.bacc as bacc
    import concourse.mybir as mybir
    from concourse import tile
    from concourse.ap import AP as _AP

    CLAMP01_ADDSC, SCALE_SUB_SQ = _register_ops()

    bf16 = mybir.dt.bfloat16
    f32 = mybir.dt.float32
    Copy = mybir.ActivationFunctionType.Copy
    Alu = mybir.AluOpType

    strips = _strips()
    NS = len(strips)

    nc = bacc.Bacc(None, target_bir_lowering=False)
    dI = nc.dram_tensor("I", [H, W], f32, kind="ExternalInput")
    dp = nc.dram_tensor("p", [3, H, W], f32, kind="ExternalInput")
    db7 = nc.dram_tensor("band7", [128, 134], bf16, kind="ExternalInput")
    dbm_f = nc.dram_tensor("bandm_first", [125, 128], bf16, kind="ExternalInput")
    dbm_i = nc.dram_tensor("bandm_int", [128, 128], bf16, kind="ExternalInput")
    dbm_l = nc.dram_tensor("bandm_last", [51, 128], bf16, kind="ExternalInput")
    dq = nc.dram_tensor("q", [3, H, W], f32, kind="ExternalOutput")

    with tile.TileContext(nc) as tc:
        with (
            tc.tile_pool(name="const", bufs=1) as constp,
            tc.tile_pool(name="resid", bufs=1) as residp,
            tc.tile_pool(name="pbuf", bufs=2) as pbufp,
            tc.tile_pool(name="prod", bufs=4) as prodp,
            tc.tile_pool(name="uirv", bufs=1) as uirvp,
            tc.tile_pool(name="ab", bufs=15) as abp,
            tc.tile_pool(name="brg", bufs=4) as brgp,
            tc.tile_pool(name="upv", bufs=2) as upvp,
            tc.tile_pool(name="f32tmp", bufs=2) as f32p,
            tc.tile_pool(name="mtmp", bufs=6) as mtmpp,
            tc.tile_pool(name="qm", bufs=2) as qmp,
            tc.tile_pool(name="psA", bufs=2, space="PSUM") as psA,
            tc.tile_pool(name="psB", bufs=2, space="PSUM") as psB,
        ):
            band7 = constp.tile([128, 134], bf16, tag="band7")
            nc.sync.dma_start(band7[:], db7.ap()[:])
            bm_first = constp.tile([125, 128], bf16, tag="bmf")
            nc.sync.dma_start(bm_first[:], dbm_f.ap()[:])
            bm_int = constp.tile([128, 128], bf16, tag="bmi")
            nc.sync.dma_start(bm_int[:], dbm_i.ap()[:])
            bm_last = constp.tile([51, 128], bf16, tag="bml")
            nc.sync.dma_start(bm_last[:], dbm_l.ap()[:])

            # Resident inputs, cast f32->bf16 during SWDGE DMA. Block layout:
            # tile[pp, i*1024 + w] = X[i*128 + pp, w].
            I_bf = residp.tile([128, 8 * 1024], bf16, tag="I_bf")
            nc.gpsimd.dma_start(
                I_bf[:].rearrange("p (i w) -> p i w", w=1024),
                dI.ap().rearrange("(i p) w -> p i w", p=128),
            )
            # 122-stride window layout for the final combine:
            # I_nat[j, m*1024 + w] = I[122m + j, w]
            I_nat = residp.tile([128, NS * 1024], bf16, tag="I_nat")
            src = dI.ap()
            src_ov = _AP(src.tensor, 0, [[1024, 128], [V * 1024, 8], [1, 1024]])
            nc.gpsimd.dma_start(
                I_nat[:, 0 : 8 * 1024].rearrange("p (m w) -> p m w", w=1024), src_ov
            )
            nc.gpsimd.dma_start(
                I_nat[0 : H - 8 * V, 8 * 1024 : 9 * 1024], dI.ap()[8 * V : H, :]
            )

            p_tiles = {}

            def load_p(c):
                t = pbufp.tile([128, 8 * 1024], bf16, tag="p_bf")
                nc.gpsimd.dma_start(
                    t[:].rearrange("p (i w) -> p i w", w=1024),
                    dp.ap()[c].rearrange("(i p) w -> p i w", p=128),
                )
                p_tiles[c] = t

            load_p(0)
            load_p(1)

            def bandm_for(si):
                if si == 0:
                    return bm_first
                if si == NS - 1:
                    return bm_last
                return bm_int

            # ---------------- matmul pass emitters ----------------
            def swap_pass(ps, tile_, off0, stride, Mw):
                """transpose+V-box: image chunks stationary, band streams.
                Accumulates 8 h-blocks into ps[0:Mw, 0:1024]."""
                seen = set()
                for i in range(8):
                    lhsT = tile_[:, i * stride + off0 : i * stride + off0 + Mw]
                    base = 128 * i - 3
                    w_lo_ = max(0, 128 * i - 3)
                    w_hi_ = min(1024, 128 * i + 131)
                    for s_lo, s_hi in _seg512(w_lo_, w_hi_):
                        bank = s_lo // 512
                        nc.tensor.matmul(
                            ps[0:Mw, s_lo:s_hi],
                            lhsT,
                            band7[:, s_lo - base : s_hi - base],
                            start=bank not in seen,
                            stop=True,
                        )
                        seen.add(bank)

            def keep_pass(ps, bridge, si):
                """H-box, band stationary: ps[0:K_out, 0:1024]"""
                i_lo, i_hi, o_lo, o_hi = strips[si]
                K = i_hi - i_lo
                bm = bandm_for(si)
                for s_lo, s_hi in _seg512(0, 1024):
                    nc.tensor.matmul(
                        ps[:, s_lo:s_hi],
                        bm[:],
                        bridge[0:K, s_lo:s_hi],
                        start=True,
                        stop=True,
                    )

            def b1_pass(ps, tiles_, m_lo, m_hi):
                """H-box of a/b over w'-strips; out ps[0:(m_hi-m_lo), 0:1024] (N)"""
                seen = set()
                for sj, (ji_lo, ji_hi, jo_lo, jo_hi) in enumerate(strips):
                    K = jo_hi - jo_lo
                    lhsT = tiles_[sj][0:K, m_lo:m_hi]
                    base = jo_lo - 3
                    w_lo_ = max(0, jo_lo - 3)
                    w_hi_ = min(1024, jo_lo + 125)
                    for s_lo, s_hi in _seg512(w_lo_, w_hi_):
                        bank = s_lo // 512
                        nc.tensor.matmul(
                            ps[0 : m_hi - m_lo, s_lo:s_hi],
                            lhsT,
                            band7[0:K, s_lo - base : s_hi - base],
                            start=bank not in seen,
                            stop=True,
                        )
                        seen.add(bank)

            I3 = I_bf[:].rearrange("p (i w) -> p i w", w=1024)

            uI_T = uirvp.tile([128, NS * 1024], bf16, tag="uI_T")
            rv_T = uirvp.tile([128, NS * 1024], bf16, tag="rv_T")

            # products: prefetched one strip ahead, on Pool
            def emit_prod_ii(s):
                i_lo, i_hi, _, _ = strips[s]
                Mw = i_hi - i_lo
                t = prodp.tile([128, 8 * 134], bf16, tag="prod")
                nc.gpsimd.tensor_mul(
                    t[:, 0 : 8 * Mw].rearrange("p (i w) -> p i w", w=Mw),
                    I3[:, :, i_lo:i_hi],
                    I3[:, :, i_lo:i_hi],
                )
                return t

            def emit_prod_ip(c, s):
                i_lo, i_hi, _, _ = strips[s]
                Mw = i_hi - i_lo
                p3 = p_tiles[c][:].rearrange("p (i w) -> p i w", w=1024)
                t = prodp.tile([128, 8 * 134], bf16, tag="prod")
                nc.gpsimd.tensor_mul(
                    t[:, 0 : 8 * Mw].rearrange("p (i w) -> p i w", w=Mw),
                    I3[:, :, i_lo:i_hi],
                    p3[:, :, i_lo:i_hi],
                )
                return t

            # ---------------- stage-1 strip blocks ----------------
            def block_A(s, ii_t, ii_next):
                """I & II boxes -> uI_T, rv_T for strip s. ii_t: product for s."""
                i_lo, i_hi, o_lo, o_hi = strips[s]
                Mw = i_hi - i_lo
                K_out = o_hi - o_lo

                psa = psA.tile([128, 1024], f32, tag="psa")
                swap_pass(psa, I_bf, i_lo, 1024, Mw)
                psa2 = psA.tile([128, 1024], f32, tag="psa")
                swap_pass(psa2, ii_t, 0, Mw, Mw)
                v1 = brgp.tile([128, 1024], bf16, tag="brg")
                nc.scalar.activation(v1[0:Mw, :], psa[0:Mw, :], Copy, bias=0.0, scale=1.0)
                v2 = brgp.tile([128, 1024], bf16, tag="brg")
                nc.scalar.activation(v2[0:Mw, :], psa2[0:Mw, :], Copy, bias=0.0, scale=1.0)
                psu = psB.tile([128, 1024], f32, tag="psb")
                keep_pass(psu, v1, s)
                psu2 = psB.tile([128, 1024], f32, tag="psb")
                keep_pass(psu2, v2, s)
                uI = uI_T[:, s * 1024 : (s + 1) * 1024]
                nc.scalar.activation(uI[0:K_out, :], psu[0:K_out, :], Copy, bias=0.0, scale=S)
                var_t = f32p.tile([128, 1024], f32, tag="f32")
                nc.vector._custom_dve(
                    SCALE_SUB_SQ,
                    out=var_t[0:K_out, :],
                    in0=psu2[0:K_out, :],
                    in1=uI[0:K_out, :],
                    s0=S,
                )
                rv32 = f32p.tile([128, 1024], f32, tag="f32")
                nc.vector.reciprocal_approx_fast(rv32[0:K_out, :], var_t[0:K_out, :])
                rv = rv_T[:, s * 1024 : (s + 1) * 1024]
                nc.scalar.activation(rv[0:K_out, :], rv32[0:K_out, :], Copy, bias=0.0, scale=1.0)

            def block_P(c, s, ip_t, a_tiles, b_tiles):
                """p_c & Ip_c boxes + a/b chain for strip s."""
                i_lo, i_hi, o_lo, o_hi = strips[s]
                Mw = i_hi - i_lo
                K_out = o_hi - o_lo
                uI = uI_T[:, s * 1024 : (s + 1) * 1024]
                rv = rv_T[:, s * 1024 : (s + 1) * 1024]

                psa = psA.tile([128, 1024], f32, tag="psa")
                swap_pass(psa, p_tiles[c], i_lo, 1024, Mw)
                psa2 = psA.tile([128, 1024], f32, tag="psa")
                swap_pass(psa2, ip_t, 0, Mw, Mw)
                v1 = brgp.tile([128, 1024], bf16, tag="brg")
                nc.scalar.activation(v1[0:Mw, :], psa[0:Mw, :], Copy, bias=0.0, scale=1.0)
                v2 = brgp.tile([128, 1024], bf16, tag="brg")
                nc.scalar.activation(v2[0:Mw, :], psa2[0:Mw, :], Copy, bias=0.0, scale=1.0)
                psu = psB.tile([128, 1024], f32, tag="psb")
                keep_pass(psu, v1, s)
                psu2 = psB.tile([128, 1024], f32, tag="psb")
                keep_pass(psu2, v2, s)

                up = upvp.tile([128, 1024], bf16, tag="up")
                nc.scalar.activation(up[0:K_out, :], psu[0:K_out, :], Copy, bias=0.0, scale=S)
                m1 = mtmpp.tile([128, 1024], bf16, tag="mt")
                nc.vector.tensor_mul(m1[0:K_out, :], uI[0:K_out, :], up[0:K_out, :])
                cov = mtmpp.tile([128, 1024], bf16, tag="mt")
                nc.vector.scalar_tensor_tensor(
                    cov[0:K_out, :], psu2[0:K_out, :], S, m1[0:K_out, :],
                    Alu.mult, Alu.subtract,
                )
                a_t = abp.tile([128, 1024], bf16, tag="a")
                nc.vector.tensor_mul(a_t[0:K_out, :], cov[0:K_out, :], rv[0:K_out, :])
                m2 = mtmpp.tile([128, 1024], bf16, tag="mt")
                nc.vector.tensor_mul(m2[0:K_out, :], a_t[0:K_out, :], uI[0:K_out, :])
                b_t = abp.tile([128, 1024], bf16, tag="b")
                nc.vector.tensor_sub(b_t[0:K_out, :], up[0:K_out, :], m2[0:K_out, :])
                a_tiles.append(a_t)
                b_tiles.append(b_t)

            # ---------------- stage-2 m blocks ----------------
            def block_S2(c, m, a_tiles, b_tiles):
                mi_lo, mi_hi, mo_lo, mo_hi = strips[m]
                Mi = mi_hi - mi_lo
                Hw = mo_hi - mo_lo

                psc_a = psA.tile([128, 1024], f32, tag="psa")
                b1_pass(psc_a, a_tiles, mi_lo, mi_hi)
                psc_b = psA.tile([128, 1024], f32, tag="psa")
                b1_pass(psc_b, b_tiles, mi_lo, mi_hi)
                # pure copies: S folded into the final fused clamp
                A_t = brgp.tile([128, 1024], bf16, tag="brg")
                nc.scalar.activation(A_t[0:Mi, :], psc_a[0:Mi, :], Copy, bias=0.0, scale=1.0)
                B_t = brgp.tile([128, 1024], bf16, tag="brg")
                nc.scalar.activation(B_t[0:Mi, :], psc_b[0:Mi, :], Copy, bias=0.0, scale=1.0)

                psd_a = psB.tile([128, 1024], f32, tag="psb")
                keep_pass(psd_a, A_t, m)
                psd_b = psB.tile([128, 1024], f32, tag="psb")
                keep_pass(psd_b, B_t, m)

                q1 = mtmpp.tile([128, 1024], bf16, tag="mt")
                nc.vector.tensor_mul(
                    q1[0:Hw, :], psd_a[0:Hw, :], I_nat[0:Hw, m * 1024 : (m + 1) * 1024]
                )
                q_t = qmp.tile([128, 1024], bf16, tag="qm")
                nc.vector._custom_dve(
                    CLAMP01_ADDSC,
                    out=q_t[0:Hw, :],
                    in0=q1[0:Hw, :],
                    in1=psd_b[0:Hw, :],
                    s0=S,
                )
                nc.gpsimd.dma_start(dq.ap()[c][mo_lo:mo_hi, :], q_t[0:Hw, :])

            # ---------------- emission schedule ----------------
            LOOKAHEAD = 6  # ab pool = 9 + 6

            with nc.named_scope("phase1"):
                ii_t = emit_prod_ii(0)
                ip_t = emit_prod_ip(0, 0)
                ab0_a, ab0_b = [], []
                for s in range(NS):
                    ii_next = emit_prod_ii(s + 1) if s + 1 < NS else None
                    ip_next = emit_prod_ip(0, s + 1) if s + 1 < NS else None
                    block_A(s, ii_t, ii_next)
                    block_P(0, s, ip_t, ab0_a, ab0_b)
                    ii_t, ip_t = ii_next, ip_next

            ab_saved = {0: (ab0_a, ab0_b)}
            for c in (0, 1):
                with nc.named_scope(f"c{c}"):
                    if c == 0:
                        load_p(2)
                    a_next, b_next = [], []
                    ip_t = emit_prod_ip(c + 1, 0)
                    for m in range(NS):
                        block_S2(c, m, *ab_saved[c])
                        sn = m - (NS - 1 - LOOKAHEAD)  # strips 0..5 during S2
                        if 0 <= sn < LOOKAHEAD:
                            ip_next = emit_prod_ip(c + 1, sn + 1)
                            block_P(c + 1, sn, ip_t, a_next, b_next)
                            ip_t = ip_next
                    for sn in range(LOOKAHEAD, NS):
                        ip_next = emit_prod_ip(c + 1, sn + 1) if sn + 1 < NS else None
                        block_P(c + 1, sn, ip_t, a_next, b_next)
                        ip_t = ip_next
                    ab_saved[c + 1] = (a_next, b_next)

            with nc.named_scope("c2"):
                for m in range(NS):
                    block_S2(2, m, *ab_saved[2])

    nc.compile()
    return nc


def kernel(I, p, radius):
    assert int(radius) == R
    I = np.ascontiguousarray(np.asarray(I, np.float32))
    p = np.ascontiguousarray(np.asarray(p, np.float32))
    B = I.shape[0]
    assert I.shape == (B, 1, H, W) and p.shape == (B, 3, H, W)

    if "nc" not in _cache:
        _cache["nc"] = _build()
    nc = _cache["nc"]

    from concourse.bass_utils import run_bass_kernel_spmd

    b7 = _band7_np()
    strips = _strips()
    bm_f = _bandm_np(*strips[0])
    bm_i = _bandm_np(*strips[1])
    bm_l = _bandm_np(*strips[-1])

    in_maps = []
    for i in range(B):
        in_maps.append(
            {
                "I": I[i, 0],
                "p": p[i],
                "band7": b7,
                "bandm_first": bm_f,
                "bandm_int": bm_i,
                "bandm_last": bm_l,
            }
        )
    res = run_bass_kernel_spmd(nc, in_maps, core_ids=list(range(B)))
    out = np.stack([res.results[i]["q"] for i in range(B)], axis=0)
    return out.astype(np.float32)


# revision 8
# speedup vs baseline: 1.0569x; 1.0360x over previous
"""Guided filter (radius=3) on 8x TRN2 NeuronCores, batch-parallel. v2.

Per core: one image. Box filters = banded matmuls on the PE (exactly one
layout-swap pass + one layout-keep pass per separable box; 28 passes/image).

v2 vs baseline:
  - Custom fused DVE ops: SCALE_SUB_SQ (var = S*uII_psum - uI^2, kills the
    separate square), CLAMP01_ADDSC (q = clip(S*(Q1*I + Q2)), kills the
    evac+add+clamp chain -- the combine reads both PSUM keeps directly).
  - Engine rebalance: ACT owns all PSUM bridge evacuations (1.11us/tile),
    Pool owns the bf16 products (I*I, I*p) + S2 B-copies + q DMA issue,
    DVE owns the PSUM-math (var/cov/recip/q-combine) + bf16 chain ops.
  - Stage-2 bridge evacs are pure copies (the 64/49 normalization is folded
    into the final fused clamp), so Pool's copy path can carry them.
  - Channel-pipelined emission: S1 of channel c+1 interleaves with S2 of
    channel c (a/b strip pools sized 15 = 9 + 6 lookahead).
"""

import sys

sys.path.insert(0, "/opt/trn_rl_repo")

import numpy as np
import ml_dtypes

R = 3
H = W = 1024
P = 128
V = 122  # valid outputs per 128-wide band matmul
S = float(64.0 / 49.0)

_cache = {}


# ---------------------------------------------------------------- custom DVE
def _register_ops():
    from concourse.dve_ops import DveOp, OPS, _SUB_OPCODE_FOR_NAME
    from concourse.dve_spec import (
        Spec, Src0, Src1, C0, Zero, One, maxx, minn, sq, lower,
    )
    from concourse.dve_uop import DveOpSpec

    def reg(name, spec):
        existing = {op.name: op for op in OPS}
        if name in existing:
            return existing[name]
        opcode = max(_SUB_OPCODE_FOR_NAME.values()) + 1
        assert opcode < 0x20, "custom-DVE opcode table full"
        shas = {}
        for ver in ("v3", "v4"):
            s = DveOpSpec(name=name, opcode=opcode, uops=lower(spec, ver=ver))
            shas[ver] = s.sha(ver)
        op = DveOp(name, spec, subdim=False, uops_sha=shas)
        OPS.append(op)
        _SUB_OPCODE_FOR_NAME[name] = opcode
        return op

    clamp = reg(
        "CLAMP01_ADDSC_ANT",
        Spec(
            body=minn(maxx((Src0 + Src1) * C0, Zero), One),
            reference=lambda in0, in1, s0, s1, imm2: np.clip(
                (in0 + in1) * s0, 0.0, 1.0
            ),
        ),
    )
    var = reg(
        "SCALE_SUB_SQ_ANT",
        Spec(
            body=Src0 * C0 - sq(Src1),
            reference=lambda in0, in1, s0, s1, imm2: in0 * s0 - in1 * in1,
        ),
    )
    return clamp, var


def _strips():
    # (in_lo, in_hi, out_lo, out_hi) along one axis
    out = []
    j = 0
    while j * V < W:
        o_lo, o_hi = j * V, min(W, j * V + V)
        i_lo, i_hi = max(0, o_lo - R), min(W, o_hi + R)
        out.append((i_lo, i_hi, o_lo, o_hi))
        j += 1
    return out


def _band7_np():
    b = np.zeros((128, 134), np.float32)
    for k in range(128):
        for d in range(134):
            if abs(d - 3 - k) <= R:
                b[k, d] = 0.125
    return b.astype(ml_dtypes.bfloat16)


def _bandm_np(i_lo, i_hi, o_lo, o_hi):
    K = i_hi - i_lo
    bm = np.zeros((K, 128), np.float32)
    for k in range(K):
        for m in range(o_hi - o_lo):
            if abs((i_lo + k) - (o_lo + m)) <= R:
                bm[k, m] = 0.125
    return bm.astype(ml_dtypes.bfloat16)


def _seg512(lo, hi):
    """split [lo,hi) at multiples of 512 (PSUM bank boundaries)"""
    segs = []
    while lo < hi:
        nxt = min(hi, (lo // 512 + 1) * 512)
        segs.append((lo, nxt))
        lo = nxt
    return segs


def _build():
    import concourse.bass as bass
    import concourse.bacc as bacc
    import concourse.mybir as mybir
    from concourse import tile
    from concourse.ap import AP as _AP

    CLAMP01_ADDSC, SCALE_SUB_SQ = _register_ops()

    bf16 = mybir.dt.bfloat16
    f32 = mybir.dt.float32
    Copy = mybir.ActivationFunctionType.Copy
    Alu = mybir.AluOpType

    strips = _strips()
    NS = len(strips)

    nc = bacc.Bacc(None, target_bir_lowering=False)
    dI = nc.dram_tensor("I", [H, W], f32, kind="ExternalInput")
    dp = nc.dram_tensor("p", [3, H, W], f32, kind="ExternalInput")
    db7 = nc.dram_tensor("band7", [128, 134], bf16, kind="ExternalInput")
    dbm_f = nc.dram_tensor("bandm_first", [125, 128], bf16, kind="ExternalInput")
    dbm_i = nc.dram_tensor("bandm_int", [128, 128], bf16, kind="ExternalInput")
    dbm_l = nc.dram_tensor("bandm_last", [51, 128], bf16, kind="ExternalInput")
    dq = nc.dram_tensor("q", [3, H, W], f32, kind="ExternalOutput")

    with tile.TileContext(nc) as tc:
        with (
            tc.tile_pool(name="const", bufs=1) as constp,
            tc.tile_pool(name="resid", bufs=1) as residp,
            tc.tile_pool(name="pbuf", bufs=2) as pbufp,
            tc.tile_pool(name="prod", bufs=4) as prodp,
            tc.tile_pool(name="uirv", bufs=1) as uirvp,
            tc.tile_pool(name="ab", bufs=15) as abp,
            tc.tile_pool(name="brg", bufs=4) as brgp,
            tc.tile_pool(name="upv", bufs=2) as upvp,
            tc.tile_pool(name="f32tmp", bufs=2) as f32p,
            tc.tile_pool(name="mtmp", bufs=6) as mtmpp,
            tc.tile_pool(name="qm", bufs=2) as qmp,
            tc.tile_pool(name="psA", bufs=2, space="PSUM") as psA,
            tc.tile_pool(name="psB", bufs=2, space="PSUM") as psB,
        ):
            band7 = constp.tile([128, 134], bf16, tag="band7")
            nc.sync.dma_start(band7[:], db7.ap()[:])
            bm_first = constp.tile([125, 128], bf16, tag="bmf")
            nc.sync.dma_start(bm_first[:], dbm_f.ap()[:])
            bm_int = constp.tile([128, 128], bf16, tag="bmi")
            nc.sync.dma_start(bm_int[:], dbm_i.ap()[:])
            bm_last = constp.tile([51, 128], bf16, tag="bml")
            nc.sync.dma_start(bm_last[:], dbm_l.ap()[:])

            # Resident inputs, cast f32->bf16 during SWDGE DMA. Block layout:
            # tile[pp, i*1024 + w] = X[i*128 + pp, w].
            I_bf = residp.tile([128, 8 * 1024], bf16, tag="I_bf")
            nc.gpsimd.dma_start(
                I_bf[:].rearrange("p (i w) -> p i w", w=1024),
                dI.ap().rearrange("(i p) w -> p i w", p=128),
            )
            # 122-stride window layout for the final combine:
            # I_nat[j, m*1024 + w] = I[122m + j, w]
            I_nat = residp.tile([128, NS * 1024], bf16, tag="I_nat")
            src = dI.ap()
            src_ov = _AP(src.tensor, 0, [[1024, 128], [V * 1024, 8], [1, 1024]])
            nc.gpsimd.dma_start(
                I_nat[:, 0 : 8 * 1024].rearrange("p (m w) -> p m w", w=1024), src_ov
            )
            nc.gpsimd.dma_start(
                I_nat[0 : H - 8 * V, 8 * 1024 : 9 * 1024], dI.ap()[8 * V : H, :]
            )

            p_tiles = {}

            def load_p(c):
                t = pbufp.tile([128, 8 * 1024], bf16, tag="p_bf")
                nc.gpsimd.dma_start(
                    t[:].rearrange("p (i w) -> p i w", w=1024),
                    dp.ap()[c].rearrange("(i p) w -> p i w", p=128),
                )
                p_tiles[c] = t

            load_p(0)
            load_p(1)

            def bandm_for(si):
                if si == 0:
                    return bm_first
                if si == NS - 1:
                    return bm_last
                return bm_int

            # ---------------- matmul pass emitters ----------------
            def swap_pass(ps, tile_, off0, stride, Mw):
                """transpose+V-box: image chunks stationary, band streams.
                Accumulates 8 h-blocks into ps[0:Mw, 0:1024]."""
                seen = set()
                for i in range(8):
                    lhsT = tile_[:, i * stride + off0 : i * stride + off0 + Mw]
                    base = 128 * i - 3
                    w_lo_ = max(0, 128 * i - 3)
                    w_hi_ = min(1024, 128 * i + 131)
                    for s_lo, s_hi in _seg512(w_lo_, w_hi_):
                        bank = s_lo // 512
                        nc.tensor.matmul(
                            ps[0:Mw, s_lo:s_hi],
                            lhsT,
                            band7[:, s_lo - base : s_hi - base],
                            start=bank not in seen,
                            stop=True,
                        )
                        seen.add(bank)

            def keep_pass(ps, bridge, si):
                """H-box, band stationary: ps[0:K_out, 0:1024]"""
                i_lo, i_hi, o_lo, o_hi = strips[si]
                K = i_hi - i_lo
                bm = bandm_for(si)
                for s_lo, s_hi in _seg512(0, 1024):
                    nc.tensor.matmul(
                        ps[:, s_lo:s_hi],
                        bm[:],
                        bridge[0:K, s_lo:s_hi],
                        start=True,
                        stop=True,
                    )

            def b1_pass(ps, tiles_, m_lo, m_hi):
                """H-box of a/b over w'-strips; out ps[0:(m_hi-m_lo), 0:1024] (N)"""
                seen = set()
                for sj, (ji_lo, ji_hi, jo_lo, jo_hi) in enumerate(strips):
                    K = jo_hi - jo_lo
                    lhsT = tiles_[sj][0:K, m_lo:m_hi]
                    base = jo_lo - 3
                    w_lo_ = max(0, jo_lo - 3)
                    w_hi_ = min(1024, jo_lo + 125)
                    for s_lo, s_hi in _seg512(w_lo_, w_hi_):
                        bank = s_lo // 512
                        nc.tensor.matmul(
                            ps[0 : m_hi - m_lo, s_lo:s_hi],
                            lhsT,
                            band7[0:K, s_lo - base : s_hi - base],
                            start=bank not in seen,
                            stop=True,
                        )
                        seen.add(bank)

            I3 = I_bf[:].rearrange("p (i w) -> p i w", w=1024)

            uI_T = uirvp.tile([128, NS * 1024], bf16, tag="uI_T")
            rv_T = uirvp.tile([128, NS * 1024], bf16, tag="rv_T")

            # products: prefetched one strip ahead, on Pool
            def emit_prod_ii(s):
                i_lo, i_hi, _, _ = strips[s]
                Mw = i_hi - i_lo
                t = prodp.tile([128, 8 * 134], bf16, tag="prod")
                nc.gpsimd.tensor_mul(
                    t[:, 0 : 8 * Mw].rearrange("p (i w) -> p i w", w=Mw),
                    I3[:, :, i_lo:i_hi],
                    I3[:, :, i_lo:i_hi],
                )
                return t

            def emit_prod_ip(c, s):
                i_lo, i_hi, _, _ = strips[s]
                Mw = i_hi - i_lo
                p3 = p_tiles[c][:].rearrange("p (i w) -> p i w", w=1024)
                t = prodp.tile([128, 8 * 134], bf16, tag="prod")
                nc.gpsimd.tensor_mul(
                    t[:, 0 : 8 * Mw].rearrange("p (i w) -> p i w", w=Mw),
                    I3[:, :, i_lo:i_hi],
                    p3[:, :, i_lo:i_hi],
                )
                return t

            # ------- elementwise tails (SBUF-only, engine-flexible) -------
            def s1_tail(c, s, psu, psu2, a_tiles, b_tiles):
                """up evac + a/b chain for strip s, channel c."""
                i_lo, i_hi, o_lo, o_hi = strips[s]
                K_out = o_hi - o_lo
                uI = uI_T[:, s * 1024 : (s + 1) * 1024]
                rv = rv_T[:, s * 1024 : (s + 1) * 1024]
                up = upvp.tile([128, 1024], bf16, tag="up")
                nc.scalar.activation(up[0:K_out, :], psu[0:K_out, :], Copy, bias=0.0, scale=S)
                m1 = mtmpp.tile([128, 1024], bf16, tag="mt")
                nc.vector.tensor_mul(m1[0:K_out, :], uI[0:K_out, :], up[0:K_out, :])
                cov = mtmpp.tile([128, 1024], bf16, tag="mt")
                nc.vector.scalar_tensor_tensor(
                    cov[0:K_out, :], psu2[0:K_out, :], S, m1[0:K_out, :],
                    Alu.mult, Alu.subtract,
                )
                a_t = abp.tile([128, 1024], bf16, tag="a")
                nc.vector.tensor_mul(a_t[0:K_out, :], cov[0:K_out, :], rv[0:K_out, :])
                m2 = mtmpp.tile([128, 1024], bf16, tag="mt")
                nc.vector.tensor_mul(m2[0:K_out, :], a_t[0:K_out, :], uI[0:K_out, :])
                b_t = abp.tile([128, 1024], bf16, tag="b")
                nc.vector.tensor_sub(b_t[0:K_out, :], up[0:K_out, :], m2[0:K_out, :])
                a_tiles.append(a_t)
                b_tiles.append(b_t)

            def s2_combine(c, m, psd_a, psd_b):
                mi_lo, mi_hi, mo_lo, mo_hi = strips[m]
                Hw = mo_hi - mo_lo
                q1 = mtmpp.tile([128, 1024], bf16, tag="mt")
                nc.vector.tensor_mul(
                    q1[0:Hw, :], psd_a[0:Hw, :], I_nat[0:Hw, m * 1024 : (m + 1) * 1024]
                )
                q_t = qmp.tile([128, 1024], bf16, tag="qm")
                nc.vector._custom_dve(
                    CLAMP01_ADDSC,
                    out=q_t[0:Hw, :],
                    in0=q1[0:Hw, :],
                    in1=psd_b[0:Hw, :],
                    s0=S,
                )
                nc.gpsimd.dma_start(dq.ap()[c][mo_lo:mo_hi, :], q_t[0:Hw, :])

            def evac(ps, rows, scale=1.0):
                t = brgp.tile([128, 1024], bf16, tag="brg")
                nc.scalar.activation(t[0:rows, :], ps[0:rows, :], Copy, bias=0.0, scale=scale)
                return t

            # -------- fused iteration: S2(c,m) micro-interleaved with S1(c1,s)
            # PE always has independent work between dependent steps.
            def iter_fused(s2=None, s1=None, prod_next=None):
                # s2 = (c, m, a_tiles, b_tiles); s1 = (c1, s, ip_t, a_next, b_next)
                if s2 is not None:
                    c, m, a_tiles, b_tiles = s2
                    mi_lo, mi_hi, mo_lo, mo_hi = strips[m]
                    Mi = mi_hi - mi_lo
                    psc_a = psA.tile([128, 1024], f32, tag="psa")
                    b1_pass(psc_a, a_tiles, mi_lo, mi_hi)
                    psc_b = psA.tile([128, 1024], f32, tag="psa")
                    b1_pass(psc_b, b_tiles, mi_lo, mi_hi)
                    A_t = evac(psc_a, Mi)
                    B_t = evac(psc_b, Mi)
                if prod_next is not None:
                    prod_next()
                if s1 is not None:
                    c1, s, ip_t, a_next, b_next = s1
                    i_lo, i_hi, o_lo, o_hi = strips[s]
                    Mw = i_hi - i_lo
                    psa = psA.tile([128, 1024], f32, tag="psa")
                    swap_pass(psa, p_tiles[c1], i_lo, 1024, Mw)
                    psa2 = psA.tile([128, 1024], f32, tag="psa")
                    swap_pass(psa2, ip_t, 0, Mw, Mw)
                if s2 is not None:
                    psd_a = psB.tile([128, 1024], f32, tag="psb")
                    keep_pass(psd_a, A_t, m)
                    psd_b = psB.tile([128, 1024], f32, tag="psb")
                    keep_pass(psd_b, B_t, m)
                if s1 is not None:
                    v1 = evac(psa, Mw)
                    v2 = evac(psa2, Mw)
                if s2 is not None:
                    s2_combine(c, m, psd_a, psd_b)
                if s1 is not None:
                    psu = psB.tile([128, 1024], f32, tag="psb")
                    keep_pass(psu, v1, s)
                    psu2 = psB.tile([128, 1024], f32, tag="psb")
                    keep_pass(psu2, v2, s)
                    s1_tail(c1, s, psu, psu2, a_next, b_next)

            # -------- phase-1 strip block: I/II + p0/Ip0 interleaved
            def iter_phase1(s, ii_t, ip_t):
                i_lo, i_hi, o_lo, o_hi = strips[s]
                Mw = i_hi - i_lo
                K_out = o_hi - o_lo
                psa = psA.tile([128, 1024], f32, tag="psa")
                swap_pass(psa, I_bf, i_lo, 1024, Mw)
                psa2 = psA.tile([128, 1024], f32, tag="psa")
                swap_pass(psa2, ii_t, 0, Mw, Mw)
                v1 = evac(psa, Mw)
                v2 = evac(psa2, Mw)
                nxt = []
                if s + 1 < NS:
                    nxt = [emit_prod_ii(s + 1), emit_prod_ip(0, s + 1)]
                psu = psB.tile([128, 1024], f32, tag="psb")
                keep_pass(psu, v1, s)
                psu2 = psB.tile([128, 1024], f32, tag="psb")
                keep_pass(psu2, v2, s)
                psa3 = psA.tile([128, 1024], f32, tag="psa")
                swap_pass(psa3, p_tiles[0], i_lo, 1024, Mw)
                psa4 = psA.tile([128, 1024], f32, tag="psa")
                swap_pass(psa4, ip_t, 0, Mw, Mw)
                uI = uI_T[:, s * 1024 : (s + 1) * 1024]
                nc.scalar.activation(uI[0:K_out, :], psu[0:K_out, :], Copy, bias=0.0, scale=S)
                var_t = f32p.tile([128, 1024], f32, tag="f32")
                nc.vector._custom_dve(
                    SCALE_SUB_SQ,
                    out=var_t[0:K_out, :],
                    in0=psu2[0:K_out, :],
                    in1=uI[0:K_out, :],
                    s0=S,
                )
                v3 = evac(psa3, Mw)
                v4 = evac(psa4, Mw)
                rv32 = f32p.tile([128, 1024], f32, tag="f32")
                nc.vector.reciprocal_approx_fast(rv32[0:K_out, :], var_t[0:K_out, :])
                psu3 = psB.tile([128, 1024], f32, tag="psb")
                keep_pass(psu3, v3, s)
                psu4 = psB.tile([128, 1024], f32, tag="psb")
                keep_pass(psu4, v4, s)
                rv = rv_T[:, s * 1024 : (s + 1) * 1024]
                nc.scalar.activation(rv[0:K_out, :], rv32[0:K_out, :], Copy, bias=0.0, scale=1.0)
                return nxt, psu3, psu4

            # ---------------- emission schedule ----------------
            LOOKAHEAD = 6  # ab pool = 9 + 6

            with nc.named_scope("phase1"):
                ii_t = emit_prod_ii(0)
                ip_t = emit_prod_ip(0, 0)
                ab0_a, ab0_b = [], []
                for s in range(NS):
                    nxt, psu3, psu4 = iter_phase1(s, ii_t, ip_t)
                    s1_tail(0, s, psu3, psu4, ab0_a, ab0_b)
                    if nxt:
                        ii_t, ip_t = nxt

            ab_saved = {0: (ab0_a, ab0_b)}
            for c in (0, 1):
                with nc.named_scope(f"c{c}"):
                    if c == 0:
                        load_p(2)
                    a_next, b_next = [], []
                    state = {"t": emit_prod_ip(c + 1, 0)}

                    def mk_prod(sn):
                        def f():
                            state["t2"] = emit_prod_ip(c + 1, sn + 1)
                        return f if sn + 1 < NS else None

                    for m in range(NS):
                        sn = m - (NS - 1 - LOOKAHEAD)  # strips 0..5 during S2
                        if 0 <= sn < LOOKAHEAD:
                            iter_fused(
                                s2=(c, m, *ab_saved[c]),
                                s1=(c + 1, sn, state["t"], a_next, b_next),
                                prod_next=mk_prod(sn),
                            )
                            state["t"] = state.get("t2")
                        else:
                            iter_fused(s2=(c, m, *ab_saved[c]))
                    for sn in range(LOOKAHEAD, NS):
                        iter_fused(
                            s1=(c + 1, sn, state["t"], a_next, b_next),
                            prod_next=mk_prod(sn),
                        )
                        state["t"] = state.get("t2")
                    ab_saved[c + 1] = (a_next, b_next)

            # c2: software-pipeline S2 blocks 2-deep (b1s of m+1 before keeps of m)
            with nc.named_scope("c2"):
                a2, b2 = ab_saved[2]
                pend = None  # (m, A_t, B_t)
                for m in range(NS):
                    mi_lo, mi_hi, mo_lo, mo_hi = strips[m]
                    Mi = mi_hi - mi_lo
                    psc_a = psA.tile([128, 1024], f32, tag="psa")
                    b1_pass(psc_a, a2, mi_lo, mi_hi)
                    psc_b = psA.tile([128, 1024], f32, tag="psa")
                    b1_pass(psc_b, b2, mi_lo, mi_hi)
                    A_t = evac(psc_a, Mi)
                    B_t = evac(psc_b, Mi)
                    if pend is not None:
                        pm, pA, pB = pend
                        psd_a = psB.tile([128, 1024], f32, tag="psb")
                        keep_pass(psd_a, pA, pm)
                        psd_b = psB.tile([128, 1024], f32, tag="psb")
                        keep_pass(psd_b, pB, pm)
                        s2_combine(2, pm, psd_a, psd_b)
                    pend = (m, A_t, B_t)
                pm, pA, pB = pend
                psd_a = psB.tile([128, 1024], f32, tag="psb")
                keep_pass(psd_a, pA, pm)
                psd_b = psB.tile([128, 1024], f32, tag="psb")
                keep_pass(psd_b, pB, pm)
                s2_combine(2, pm, psd_a, psd_b)

    nc.compile()
    return nc


def kernel(I, p, radius):
    assert int(radius) == R
    I = np.ascontiguousarray(np.asarray(I, np.float32))
    p = np.ascontiguousarray(np.asarray(p, np.float32))
    B = I.shape[0]
    assert I.shape == (B, 1, H, W) and p.shape == (B, 3, H, W)

    if "nc" not in _cache:
        _cache["nc"] = _build()
    nc = _cache["nc"]

    from concourse.bass_utils import run_bass_kernel_spmd

    b7 = _band7_np()
    strips = _strips()
    bm_f = _bandm_np(*strips[0])
    bm_i = _bandm_np(*strips[1])
    bm_l = _bandm_np(*strips[-1])

    in_maps = []
    for i in range(B):
        in_maps.append(
            {
                "I": I[i, 0],
                "p": p[i],
                "band7": b7,
                "bandm_first": bm_f,
                "bandm_int": bm_i,
                "bandm_last": bm_l,
            }
        )
    res = run_bass_kernel_spmd(nc, in_maps, core_ids=list(range(B)))
    out = np.stack([res.results[i]["q"] for i in range(B)], axis=0)
    return out.astype(np.float32)


# revision 10
# speedup vs baseline: 1.0942x; 1.0353x over previous
"""Guided filter (radius=3) on 8x TRN2 NeuronCores, batch-parallel. v2.

Per core: one image. Box filters = banded matmuls on the PE (exactly one
layout-swap pass + one layout-keep pass per separable box; 28 passes/image).

v2 vs baseline:
  - Custom fused DVE ops: SCALE_SUB_SQ (var = S*uII_psum - uI^2, kills the
    separate square), CLAMP01_ADDSC (q = clip(S*(Q1*I + Q2)), kills the
    evac+add+clamp chain -- the combine reads both PSUM keeps directly).
  - Engine rebalance: ACT owns all PSUM bridge evacuations (1.11us/tile),
    Pool owns the bf16 products (I*I, I*p) + S2 B-copies + q DMA issue,
    DVE owns the PSUM-math (var/cov/recip/q-combine) + bf16 chain ops.
  - Stage-2 bridge evacs are pure copies (the 64/49 normalization is folded
    into the final fused clamp), so Pool's copy path can carry them.
  - Channel-pipelined emission: S1 of channel c+1 interleaves with S2 of
    channel c (a/b strip pools sized 15 = 9 + 6 lookahead).
"""

import sys

sys.path.insert(0, "/opt/trn_rl_repo")

import numpy as np
import ml_dtypes

R = 3
H = W = 1024
P = 128
V = 122  # valid outputs per 128-wide band matmul
S = float(64.0 / 49.0)

_cache = {}


# ---------------------------------------------------------------- custom DVE
def _register_ops():
    from concourse.dve_ops import DveOp, OPS, _SUB_OPCODE_FOR_NAME
    from concourse.dve_spec import (
        Spec, Src0, Src1, C0, Zero, One, maxx, minn, sq, lower,
    )
    from concourse.dve_uop import DveOpSpec

    def reg(name, spec):
        existing = {op.name: op for op in OPS}
        if name in existing:
            return existing[name]
        opcode = max(_SUB_OPCODE_FOR_NAME.values()) + 1
        assert opcode < 0x20, "custom-DVE opcode table full"
        shas = {}
        for ver in ("v3", "v4"):
            s = DveOpSpec(name=name, opcode=opcode, uops=lower(spec, ver=ver))
            shas[ver] = s.sha(ver)
        op = DveOp(name, spec, subdim=False, uops_sha=shas)
        OPS.append(op)
        _SUB_OPCODE_FOR_NAME[name] = opcode
        return op

    clamp = reg(
        "CLAMP01_ADDSC_ANT",
        Spec(
            body=minn(maxx((Src0 + Src1) * C0, Zero), One),
            reference=lambda in0, in1, s0, s1, imm2: np.clip(
                (in0 + in1) * s0, 0.0, 1.0
            ),
        ),
    )
    var = reg(
        "SCALE_SUB_SQ_ANT",
        Spec(
            body=Src0 * C0 - sq(Src1),
            reference=lambda in0, in1, s0, s1, imm2: in0 * s0 - in1 * in1,
        ),
    )
    return clamp, var


def _strips():
    # (in_lo, in_hi, out_lo, out_hi) along one axis
    out = []
    j = 0
    while j * V < W:
        o_lo, o_hi = j * V, min(W, j * V + V)
        i_lo, i_hi = max(0, o_lo - R), min(W, o_hi + R)
        out.append((i_lo, i_hi, o_lo, o_hi))
        j += 1
    return out


def _band7_np():
    b = np.zeros((128, 134), np.float32)
    for k in range(128):
        for d in range(134):
            if abs(d - 3 - k) <= R:
                b[k, d] = 0.125
    return b.astype(ml_dtypes.bfloat16)


def _bandm_np(i_lo, i_hi, o_lo, o_hi):
    K = i_hi - i_lo
    bm = np.zeros((K, 128), np.float32)
    for k in range(K):
        for m in range(o_hi - o_lo):
            if abs((i_lo + k) - (o_lo + m)) <= R:
                bm[k, m] = 0.125
    return bm.astype(ml_dtypes.bfloat16)


def _seg512(lo, hi):
    """split [lo,hi) at multiples of 512 (PSUM bank boundaries)"""
    segs = []
    while lo < hi:
        nxt = min(hi, (lo // 512 + 1) * 512)
        segs.append((lo, nxt))
        lo = nxt
    return segs


def _build():
    import concourse.bass as bass
    import concourse.bacc as bacc
    import concourse.mybir as mybir
    from concourse import tile
    from concourse.ap import AP as _AP

    CLAMP01_ADDSC, SCALE_SUB_SQ = _register_ops()

    bf16 = mybir.dt.bfloat16
    f32 = mybir.dt.float32
    f8 = mybir.dt.float8e4
    Copy = mybir.ActivationFunctionType.Copy
    Alu = mybir.AluOpType

    strips = _strips()
    NS = len(strips)

    nc = bacc.Bacc(None, target_bir_lowering=False)
    dI = nc.dram_tensor("I", [H, W], f32, kind="ExternalInput")
    dp = nc.dram_tensor("p", [3, H, W], f32, kind="ExternalInput")
    db7 = nc.dram_tensor("band7", [128, 134], bf16, kind="ExternalInput")
    dbm_f = nc.dram_tensor("bandm_first", [125, 128], bf16, kind="ExternalInput")
    dbm_i = nc.dram_tensor("bandm_int", [128, 128], bf16, kind="ExternalInput")
    dbm_l = nc.dram_tensor("bandm_last", [51, 128], bf16, kind="ExternalInput")
    dq = nc.dram_tensor("q", [3, H, W], f32, kind="ExternalOutput")

    with tile.TileContext(nc) as tc:
        with (
            tc.tile_pool(name="const", bufs=1) as constp,
            tc.tile_pool(name="resid", bufs=1) as residp,
            tc.tile_pool(name="pbuf", bufs=2) as pbufp,
            tc.tile_pool(name="prod", bufs=3) as prodp,
            tc.tile_pool(name="uirv", bufs=1) as uirvp,
            tc.tile_pool(name="ab", bufs=18) as abp,
            tc.tile_pool(name="brg", bufs=4) as brgp,
            tc.tile_pool(name="upv", bufs=2) as upvp,
            tc.tile_pool(name="f32tmp", bufs=2) as f32p,
            tc.tile_pool(name="mtmp", bufs=5) as mtmpp,
            tc.tile_pool(name="qm", bufs=2) as qmp,
            tc.tile_pool(name="psA", bufs=2, space="PSUM") as psA,
            tc.tile_pool(name="psB", bufs=2, space="PSUM") as psB,
        ):
            band7 = constp.tile([128, 134], bf16, tag="band7")
            nc.sync.dma_start(band7[:], db7.ap()[:])
            band7_f8 = constp.tile([128, 134], f8, tag="band7f8")
            nc.vector.tensor_copy(band7_f8[:], band7[:])
            bm_first = constp.tile([125, 128], bf16, tag="bmf")
            nc.sync.dma_start(bm_first[:], dbm_f.ap()[:])
            bm_int = constp.tile([128, 128], bf16, tag="bmi")
            nc.sync.dma_start(bm_int[:], dbm_i.ap()[:])
            bm_last = constp.tile([51, 128], bf16, tag="bml")
            nc.sync.dma_start(bm_last[:], dbm_l.ap()[:])

            # Resident inputs, cast f32->bf16 during SWDGE DMA. Block layout:
            # tile[pp, i*1024 + w] = X[i*128 + pp, w].
            I_bf = residp.tile([128, 8 * 1024], bf16, tag="I_bf")
            nc.gpsimd.dma_start(
                I_bf[:].rearrange("p (i w) -> p i w", w=1024),
                dI.ap().rearrange("(i p) w -> p i w", p=128),
            )

            p_tiles = {}

            def load_p(c):
                t = pbufp.tile([128, 8 * 1024], bf16, tag="p_bf")
                nc.gpsimd.dma_start(
                    t[:].rearrange("p (i w) -> p i w", w=1024),
                    dp.ap()[c].rearrange("(i p) w -> p i w", p=128),
                )
                p_tiles[c] = t

            load_p(0)

            def load_I_nat():
                # 122-stride window layout for the final combine:
                # I_nat[j, m*1024 + w] = I[122m + j, w]
                t = residp.tile([128, NS * 1024], bf16, tag="I_nat")
                src = dI.ap()
                src_ov = _AP(src.tensor, 0, [[1024, 128], [V * 1024, 8], [1, 1024]])
                nc.gpsimd.dma_start(
                    t[:, 0 : 8 * 1024].rearrange("p (m w) -> p m w", w=1024), src_ov
                )
                nc.gpsimd.dma_start(
                    t[0 : H - 8 * V, 8 * 1024 : 9 * 1024], dI.ap()[8 * V : H, :]
                )
                return t

            def bandm_for(si):
                if si == 0:
                    return bm_first
                if si == NS - 1:
                    return bm_last
                return bm_int

            # ---------------- matmul pass emitters ----------------
            def swap_pass(ps, tile_, off0, stride, Mw):
                """transpose+V-box: image chunks stationary, band streams.
                Accumulates 8 h-blocks into ps[0:Mw, 0:1024]."""
                seen = set()
                for i in range(8):
                    lhsT = tile_[:, i * stride + off0 : i * stride + off0 + Mw]
                    base = 128 * i - 3
                    w_lo_ = max(0, 128 * i - 3)
                    w_hi_ = min(1024, 128 * i + 131)
                    for s_lo, s_hi in _seg512(w_lo_, w_hi_):
                        bank = s_lo // 512
                        nc.tensor.matmul(
                            ps[0:Mw, s_lo:s_hi],
                            lhsT,
                            band7[:, s_lo - base : s_hi - base],
                            start=bank not in seen,
                            stop=True,
                        )
                        seen.add(bank)

            def keep_pass(ps, bridge, si):
                """H-box, band stationary: ps[0:K_out, 0:1024]"""
                i_lo, i_hi, o_lo, o_hi = strips[si]
                K = i_hi - i_lo
                bm = bandm_for(si)
                for s_lo, s_hi in _seg512(0, 1024):
                    nc.tensor.matmul(
                        ps[:, s_lo:s_hi],
                        bm[:],
                        bridge[0:K, s_lo:s_hi],
                        start=True,
                        stop=True,
                    )

            def b1_pass(ps, tiles_, m_lo, m_hi, band=None):
                """H-box of a/b over w'-strips; out ps[0:(m_hi-m_lo), 0:1024] (N)"""
                seen = set()
                for sj, (ji_lo, ji_hi, jo_lo, jo_hi) in enumerate(strips):
                    K = jo_hi - jo_lo
                    lhsT = tiles_[sj][0:K, m_lo:m_hi]
                    base = jo_lo - 3
                    w_lo_ = max(0, jo_lo - 3)
                    w_hi_ = min(1024, jo_lo + 125)
                    for s_lo, s_hi in _seg512(w_lo_, w_hi_):
                        bank = s_lo // 512
                        nc.tensor.matmul(
                            ps[0 : m_hi - m_lo, s_lo:s_hi],
                            lhsT,
                            (band if band is not None else band7)[0:K, s_lo - base : s_hi - base],
                            start=bank not in seen,
                            stop=True,
                        )
                        seen.add(bank)

            I3 = I_bf[:].rearrange("p (i w) -> p i w", w=1024)

            uI_T = uirvp.tile([128, NS * 1024], bf16, tag="uI_T")
            rv_T = uirvp.tile([128, NS * 1024], bf16, tag="rv_T")

            # products: prefetched one strip ahead, on Pool
            def emit_prod_ii(s):
                i_lo, i_hi, _, _ = strips[s]
                Mw = i_hi - i_lo
                t = prodp.tile([128, 8 * 134], bf16, tag="prod")
                nc.gpsimd.tensor_mul(
                    t[:, 0 : 8 * Mw].rearrange("p (i w) -> p i w", w=Mw),
                    I3[:, :, i_lo:i_hi],
                    I3[:, :, i_lo:i_hi],
                )
                return t

            def emit_prod_ip(c, s):
                i_lo, i_hi, _, _ = strips[s]
                Mw = i_hi - i_lo
                p3 = p_tiles[c][:].rearrange("p (i w) -> p i w", w=1024)
                t = prodp.tile([128, 8 * 134], bf16, tag="prod")
                nc.gpsimd.tensor_mul(
                    t[:, 0 : 8 * Mw].rearrange("p (i w) -> p i w", w=Mw),
                    I3[:, :, i_lo:i_hi],
                    p3[:, :, i_lo:i_hi],
                )
                return t

            # ------- elementwise tails (SBUF-only, engine-flexible) -------
            def s1_tail(c, s, psu, psu2, a_tiles, b_tiles):
                """up evac + a/b chain for strip s, channel c."""
                i_lo, i_hi, o_lo, o_hi = strips[s]
                K_out = o_hi - o_lo
                uI = uI_T[:, s * 1024 : (s + 1) * 1024]
                rv = rv_T[:, s * 1024 : (s + 1) * 1024]
                up = upvp.tile([128, 1024], bf16, tag="up")
                nc.scalar.activation(up[0:K_out, :], psu[0:K_out, :], Copy, bias=0.0, scale=S)
                m1 = mtmpp.tile([128, 1024], bf16, tag="mt")
                nc.vector.tensor_mul(m1[0:K_out, :], uI[0:K_out, :], up[0:K_out, :])
                cov = mtmpp.tile([128, 1024], bf16, tag="mt")
                nc.vector.scalar_tensor_tensor(
                    cov[0:K_out, :], psu2[0:K_out, :], S, m1[0:K_out, :],
                    Alu.mult, Alu.subtract,
                )
                a_t = abp.tile([128, 1024], bf16, tag="a")
                nc.vector.tensor_mul(a_t[0:K_out, :], cov[0:K_out, :], rv[0:K_out, :])
                m2 = mtmpp.tile([128, 1024], bf16, tag="mt")
                nc.vector.tensor_mul(m2[0:K_out, :], a_t[0:K_out, :], uI[0:K_out, :])
                b_t = abp.tile([128, 1024], f8, tag="b")
                nc.vector.tensor_sub(b_t[0:K_out, :], up[0:K_out, :], m2[0:K_out, :])
                a_tiles.append(a_t)
                b_tiles.append(b_t)

            def s2_combine(c, m, psd_a, psd_b):
                mi_lo, mi_hi, mo_lo, mo_hi = strips[m]
                Hw = mo_hi - mo_lo
                q1 = mtmpp.tile([128, 1024], bf16, tag="mt")
                nc.vector.tensor_mul(
                    q1[0:Hw, :], psd_a[0:Hw, :], I_nat[0:Hw, m * 1024 : (m + 1) * 1024]
                )
                q_t = qmp.tile([128, 1024], bf16, tag="qm")
                nc.vector._custom_dve(
                    CLAMP01_ADDSC,
                    out=q_t[0:Hw, :],
                    in0=q1[0:Hw, :],
                    in1=psd_b[0:Hw, :],
                    s0=S,
                )
                nc.gpsimd.dma_start(dq.ap()[c][mo_lo:mo_hi, :], q_t[0:Hw, :])

            def evac(ps, rows, scale=1.0):
                t = brgp.tile([128, 1024], bf16, tag="brg")
                nc.scalar.activation(t[0:rows, :], ps[0:rows, :], Copy, bias=0.0, scale=scale)
                return t

            # -------- fused iteration: S2(c,m) micro-interleaved with S1(c1,s)
            # PE always has independent work between dependent steps.
            def iter_fused(s2=None, s1=None, prod_next=None):
                # s2 = (c, m, a_tiles, b_tiles); s1 = (c1, s, ip_t, a_next, b_next)
                if s2 is not None:
                    c, m, a_tiles, b_tiles = s2
                    mi_lo, mi_hi, mo_lo, mo_hi = strips[m]
                    Mi = mi_hi - mi_lo
                    psc_a = psA.tile([128, 1024], f32, tag="psa")
                    b1_pass(psc_a, a_tiles, mi_lo, mi_hi)
                    psc_b = psA.tile([128, 1024], f32, tag="psa")
                    b1_pass(psc_b, b_tiles, mi_lo, mi_hi, band=band7_f8)
                    A_t = evac(psc_a, Mi)
                    B_t = evac(psc_b, Mi)
                if prod_next is not None:
                    prod_next()
                if s1 is not None:
                    c1, s, ip_t, a_next, b_next = s1
                    i_lo, i_hi, o_lo, o_hi = strips[s]
                    Mw = i_hi - i_lo
                    psa = psA.tile([128, 1024], f32, tag="psa")
                    swap_pass(psa, p_tiles[c1], i_lo, 1024, Mw)
                    psa2 = psA.tile([128, 1024], f32, tag="psa")
                    swap_pass(psa2, ip_t, 0, Mw, Mw)
                if s2 is not None:
                    psd_a = psB.tile([128, 1024], f32, tag="psb")
                    keep_pass(psd_a, A_t, m)
                    psd_b = psB.tile([128, 1024], f32, tag="psb")
                    keep_pass(psd_b, B_t, m)
                if s1 is not None:
                    v1 = evac(psa, Mw)
                    v2 = evac(psa2, Mw)
                if s2 is not None:
                    s2_combine(c, m, psd_a, psd_b)
                if s1 is not None:
                    psu = psB.tile([128, 1024], f32, tag="psb")
                    keep_pass(psu, v1, s)
                    psu2 = psB.tile([128, 1024], f32, tag="psb")
                    keep_pass(psu2, v2, s)
                    s1_tail(c1, s, psu, psu2, a_next, b_next)

            # -------- phase-1 strip block: I/II + p0/Ip0 interleaved
            def iter_phase1(s, ii_t, ip_t):
                i_lo, i_hi, o_lo, o_hi = strips[s]
                Mw = i_hi - i_lo
                K_out = o_hi - o_lo
                psa = psA.tile([128, 1024], f32, tag="psa")
                swap_pass(psa, I_bf, i_lo, 1024, Mw)
                psa2 = psA.tile([128, 1024], f32, tag="psa")
                swap_pass(psa2, ii_t, 0, Mw, Mw)
                v1 = evac(psa, Mw)
                v2 = evac(psa2, Mw)
                nxt = []
                if s + 1 < NS:
                    nxt = [emit_prod_ii(s + 1), emit_prod_ip(0, s + 1)]
                psu = psB.tile([128, 1024], f32, tag="psb")
                keep_pass(psu, v1, s)
                psu2 = psB.tile([128, 1024], f32, tag="psb")
                keep_pass(psu2, v2, s)
                psa3 = psA.tile([128, 1024], f32, tag="psa")
                swap_pass(psa3, p_tiles[0], i_lo, 1024, Mw)
                psa4 = psA.tile([128, 1024], f32, tag="psa")
                swap_pass(psa4, ip_t, 0, Mw, Mw)
                uI = uI_T[:, s * 1024 : (s + 1) * 1024]
                nc.scalar.activation(uI[0:K_out, :], psu[0:K_out, :], Copy, bias=0.0, scale=S)
                var_t = f32p.tile([128, 1024], f32, tag="f32")
                nc.vector._custom_dve(
                    SCALE_SUB_SQ,
                    out=var_t[0:K_out, :],
                    in0=psu2[0:K_out, :],
                    in1=uI[0:K_out, :],
                    s0=S,
                )
                v3 = evac(psa3, Mw)
                v4 = evac(psa4, Mw)
                rv32 = f32p.tile([128, 1024], f32, tag="f32")
                nc.vector.reciprocal_approx_fast(rv32[0:K_out, :], var_t[0:K_out, :])
                psu3 = psB.tile([128, 1024], f32, tag="psb")
                keep_pass(psu3, v3, s)
                psu4 = psB.tile([128, 1024], f32, tag="psb")
                keep_pass(psu4, v4, s)
                rv = rv_T[:, s * 1024 : (s + 1) * 1024]
                nc.scalar.activation(rv[0:K_out, :], rv32[0:K_out, :], Copy, bias=0.0, scale=1.0)
                return nxt, psu3, psu4

            # ---------------- emission schedule ----------------
            LOOKAHEAD = 9  # every S2 block pairs with an S1 strip

            with nc.named_scope("phase1"):
                ii_t = emit_prod_ii(0)
                ip_t = emit_prod_ip(0, 0)
                I_nat = load_I_nat()
                load_p(1)
                ab0_a, ab0_b = [], []
                for s in range(NS):
                    nxt, psu3, psu4 = iter_phase1(s, ii_t, ip_t)
                    s1_tail(0, s, psu3, psu4, ab0_a, ab0_b)
                    if nxt:
                        ii_t, ip_t = nxt

            ab_saved = {0: (ab0_a, ab0_b)}
            for c in (0, 1):
                with nc.named_scope(f"c{c}"):
                    if c == 0:
                        load_p(2)
                    a_next, b_next = [], []
                    state = {"t": emit_prod_ip(c + 1, 0)}

                    def mk_prod(sn):
                        def f():
                            state["t2"] = emit_prod_ip(c + 1, sn + 1)
                        return f if sn + 1 < NS else None

                    for m in range(NS):
                        iter_fused(
                            s2=(c, m, *ab_saved[c]),
                            s1=(c + 1, m, state["t"], a_next, b_next),
                            prod_next=mk_prod(m),
                        )
                        state["t"] = state.get("t2")
                    ab_saved[c + 1] = (a_next, b_next)

            # c2: software-pipeline S2 blocks 2-deep (b1s of m+1 before keeps of m)
            with nc.named_scope("c2"):
                a2, b2 = ab_saved[2]
                pend = None  # (m, A_t, B_t)
                for m in range(NS):
                    mi_lo, mi_hi, mo_lo, mo_hi = strips[m]
                    Mi = mi_hi - mi_lo
                    psc_a = psA.tile([128, 1024], f32, tag="psa")
                    b1_pass(psc_a, a2, mi_lo, mi_hi)
                    psc_b = psA.tile([128, 1024], f32, tag="psa")
                    b1_pass(psc_b, b2, mi_lo, mi_hi, band=band7_f8)
                    A_t = evac(psc_a, Mi)
                    B_t = evac(psc_b, Mi)
                    if pend is not None:
                        pm, pA, pB = pend
                        psd_a = psB.tile([128, 1024], f32, tag="psb")
                        keep_pass(psd_a, pA, pm)
                        psd_b = psB.tile([128, 1024], f32, tag="psb")
                        keep_pass(psd_b, pB, pm)
                        s2_combine(2, pm, psd_a, psd_b)
                    pend = (m, A_t, B_t)
                pm, pA, pB = pend
                psd_a = psB.tile([128, 1024], f32, tag="psb")
                keep_pass(psd_a, pA, pm)
                psd_b = psB.tile([128, 1024], f32, tag="psb")
                keep_pass(psd_b, pB, pm)
                s2_combine(2, pm, psd_a, psd_b)

    nc.compile()
    return nc


def kernel(I, p, radius):
    assert int(radius) == R
    I = np.ascontiguousarray(np.asarray(I, np.float32))
    p = np.ascontiguousarray(np.asarray(p, np.float32))
    B = I.shape[0]
    assert I.shape == (B, 1, H, W) and p.shape == (B, 3, H, W)

    if "nc" not in _cache:
        _cache["nc"] = _build()
    nc = _cache["nc"]

    from concourse.bass_utils import run_bass_kernel_spmd

    b7 = _band7_np()
    strips = _strips()
    bm_f = _bandm_np(*strips[0])
    bm_i = _bandm_np(*strips[1])
    bm_l = _bandm_np(*strips[-1])

    in_maps = []
    for i in range(B):
        in_maps.append(
            {
                "I": I[i, 0],
                "p": p[i],
                "band7": b7,
                "bandm_first": bm_f,
                "bandm_int": bm_i,
                "bandm_last": bm_l,
            }
        )
    res = run_bass_kernel_spmd(nc, in_maps, core_ids=list(range(B)))
    out = np.stack([res.results[i]["q"] for i in range(B)], axis=0)
    return out.astype(np.float32)
